# revision 2
# baseline (speedup 1.0000x reference)
"""HalfKP NNUE-style network on 8 Trainium2 NeuronCores — v2.

Launch 1 (feature transformer, F-dim sharded 8 ways):
  Each core owns a 5120-wide slice of F for BOTH colors. Host pre-transposes
  features to [f, b] tiles, fp16 (weights pre-scaled by WSCALE to stay in
  fp16 normal range). Full batch (2048) kept per f-tile: 8 matmuls of
  [128f,128h] x [128f,512b] per f-tile accumulate into all 8 PSUM banks;
  LDWEIGHTS is hidden by the PE reorder window. Features stream as 4-ftile
  2MB DMA chunks (near-peak HBM BW) on the sync+scalar HWDGE rings; first
  two ftiles are small chunks so the PE starts early; weight tails and
  partial write-outs ride the gpsimd SWDGE ring.

Host glue: sum the 8 partial tensors (fp32), re-shard by batch.

Launch 2 (tiny MLP, batch sharded): bias+ReLU then 512->32->32->1 MLP, tanh.
"""

import sys

import numpy as np

sys.path.insert(0, "/opt/trn_rl_repo")

import concourse.bass as bass
import concourse.bacc as bacc
import concourse.tile as tile
import concourse.mybir as mybir
from concourse import bass_utils

F16 = np.float16
F32 = np.float32
WSCALE = 256.0  # ft weights pre-scaled into fp16 normal range; undone in launch 2

B = 2048
F = 40960
H1 = 256
NCORES = 8
FS = F // NCORES        # features per core: 5120
NFT = FS // 128         # f-tiles per core: 40
NHT = H1 // 128         # h-tiles: 2
NCK = B // 512          # 512-wide batch chunks: 4
BSH = B // NCORES       # batch rows per core in launch 2: 256

DT_F16 = mybir.dt.float16
DT_F32 = mybir.dt.float32


def build_ft_kernel(nc):
    """partial[c, ht, p, b] = sum_f W[c][ht*128+p, f] * feat[c][b, f] over this
    core's F slice (weights pre-scaled by WSCALE)."""
    feats = nc.dram_tensor(
        "feats", [2, 128, NFT * B], DT_F16, kind="ExternalInput"
    ).ap()
    wts = nc.dram_tensor(
        "wts", [2, 128, NFT * H1], DT_F16, kind="ExternalInput"
    ).ap()
    partial = nc.dram_tensor(
        "partial", [2, 128, NHT * B], DT_F16, kind="ExternalOutput"
    ).ap()

    wcols = NFT * H1  # 10240
    AF = mybir.ActivationFunctionType

    with tile.TileContext(nc) as tc:
        with (
            tc.tile_pool(name="wpool", bufs=1) as wpool,
            tc.tile_pool(name="f2pool", bufs=4) as f2pool,
            tc.tile_pool(name="f4pool", bufs=7) as f4pool,
            tc.tile_pool(name="opool", bufs=2) as opool,
            tc.tile_pool(name="pspool", bufs=1, space=bass.MemorySpace.PSUM) as pspool,
            nc.sbuf_tensor("dW_raw", [128, 128], DT_F16) as dW_h,
            nc.sbuf_tensor("dF_raw", [128, 512], DT_F16) as dF_h,
        ):
            dW_raw = dW_h[:]
            dF_raw = dF_h[:]
            w_sb = []
            for c in range(2):
                w = wpool.tile([128, wcols], DT_F16, tag=f"w{c}", name=f"w{c}")
                w_sb.append(w)

            # ---- interleaved input schedule on the two HWDGE rings ----
            # Items listed in the order the matmul stream needs them (weight
            # slices just before the feature chunks they serve, c1 weights
            # deferred to their actual need times); each item goes to the ring
            # with fewer queued bytes so the per-ring byte prefix — and hence
            # arrival time at the shared ~358GB/s HBM rate — tracks need
            # order. All input issues precede the matmul stream in program
            # order so nothing head-of-line blocks the rings.
            # Ring pinned (r=0/1) for the critical fill phase — features split
            # across rings so each ring's byte prefix tracks need order even
            # if the rings drain unevenly; greedy byte-balance (r=None) after.
            items = [
                ("f", 0, 0, 2, 0), ("w", 0, 0, 2, 1), ("f", 0, 2, 2, 1),
                ("w", 0, 2, 6, 1), ("f", 0, 4, 2, 0), ("w", 0, 8, 8, 1),
                ("f", 0, 6, 2, 1), ("f", 0, 8, 4, 0), ("f", 0, 12, 4, 1),
                ("w", 0, 16, 12, None),
                ("f", 0, 16, 4, None), ("f", 0, 20, 4, None),
                ("f", 0, 24, 4, None), ("w", 0, 28, 12, None),
                ("f", 0, 28, 4, None), ("f", 0, 32, 4, None),
                ("f", 0, 36, 4, None),
                ("w", 1, 0, 14, None), ("f", 1, 0, 4, None),
                ("f", 1, 4, 4, None), ("w", 1, 14, 14, None),
                ("f", 1, 8, 4, None), ("f", 1, 12, 4, None),
                ("f", 1, 16, 4, None), ("w", 1, 28, 12, None),
                ("f", 1, 20, 4, None), ("f", 1, 24, 4, None),
                ("f", 1, 28, 4, None), ("f", 1, 32, 4, None),
                ("f", 1, 36, 4, None),
            ]

            pools = {2: f2pool, 4: f4pool}
            chunk_of = {}   # (c, ft) -> (tile, col offset)
            ring = [nc.sync, nc.scalar]
            ring_bytes = [0, 0]
            for kind, c, fstart, nft, rpin in items:
                if rpin is not None:
                    r = rpin
                else:
                    r = 0 if ring_bytes[0] <= ring_bytes[1] else 1
                eng = ring[r]
                if kind == "w":
                    ring_bytes[r] += nft * H1 * 128 * 2
                    eng.dma_start(
                        w_sb[c][:, fstart * H1:(fstart + nft) * H1],
                        wts[c, :, fstart * H1:(fstart + nft) * H1])
                else:
                    ring_bytes[r] += nft * B * 128 * 2
                    t = pools[nft].tile([128, nft * B], DT_F16, tag=f"f{nft}",
                                        name=f"f{nft}_{c}_{fstart}")
                    eng.dma_start(t[:], feats[c, :, fstart * B:(fstart + nft) * B])
                    for k in range(nft):
                        chunk_of[(c, fstart + k)] = (t, k * B)

            ps = [pspool.tile([128, 512], DT_F32, tag=f"ps{i}", name=f"ps{i}")
                  for i in range(8)]

            # pull the scalar ACT table load to mid-kernel (idle queue time)
            # so the tail's scalar copies don't stall on it
            warm = opool.tile([1, 1], DT_F16, tag="warm")
            nc.scalar.activation(warm[0:1, 0:1], warm[0:1, 0:1], AF.Copy)

            # HAM prewarm: dummy matmuls over raw (untracked, uninitialized)
            # SBUF run as the tensor queue's first real work (~5.5us, right
            # after its prologue) and keep the PE busy until the first feature
            # chunk lands, so the clock gate is at 8/8 — and the PE pipeline
            # hot — when the real stream starts. Garbage inputs are fine: the
            # real accumulations open with start=True which resets PSUM.
            for i in range(30):
                nc.tensor.matmul(ps[i % 2][:], dW_raw, dF_raw,
                                 start=True, stop=True)

            # ---- main matmul stream ----
            for c in range(2):
                for ft in range(NFT):
                    ftile, off = chunk_of[(c, ft)]
                    for ht in range(NHT):
                        lhsT = w_sb[c][:, ft * H1 + ht * 128:
                                       ft * H1 + (ht + 1) * 128]
                        for ck in range(NCK):
                            nc.tensor.matmul(
                                ps[ht * NCK + ck][:],
                                lhsT,
                                ftile[:, off + ck * 512: off + (ck + 1) * 512],
                                start=(ft == 0),
                                stop=(ft == NFT - 1),
                            )
                # PSUM -> SBUF casts into one [128, NHT*B] staging tile, then
                # ONE store per color (single DMA issue, ~1MB). Color 0's
                # store trickles out on the idle gpsimd ring under color 1's
                # stream; color 1's (the tail) goes on sync. Tail casts split
                # vector/scalar so they finish in ~half the time.
                ot = opool.tile([128, NHT * B], DT_F16, tag="out",
                                name=f"o{c}")
                for ht in range(NHT):
                    for ck in range(NCK):
                        i = ht * NCK + ck
                        dst = ot[:, ht * B + ck * 512: ht * B + (ck + 1) * 512]
                        if c == 0 or i < 4:
                            nc.vector.tensor_copy(dst, ps[i][:])
                        else:
                            nc.scalar.activation(dst, ps[i][:], AF.Copy)
                if c == 0:
                    nc.gpsimd.dma_start(partial[c], ot[:])
                else:
                    # two half-stores on separate rings so the tail transfer
                    # halves
                    nc.sync.dma_start(partial[c, :, 0:B], ot[:, 0:B])
                    nc.scalar.dma_start(partial[c, :, B:NHT * B],
                                        ot[:, B:NHT * B])
    return nc


def build_mlp_kernel(nc):
    """bias+relu on host-reduced pre-activations, then the MLP.

    pre[p, (c*NHT+ht)*BSH + b] = host-summed partial preact (scaled by WSCALE).
    consts packs every weight/bias into one [128, ncol] f32 tensor.
    """
    nxt = 2 * NHT
    pre = nc.dram_tensor("pre", [128, nxt * BSH], DT_F32, kind="ExternalInput").ap()
    ncol = 128 + nxt + 36
    consts = nc.dram_tensor("consts", [128, ncol], DT_F32, kind="ExternalInput").ap()
    out = nc.dram_tensor("out", [1, BSH], DT_F32, kind="ExternalOutput").ap()

    AF = mybir.ActivationFunctionType

    with tile.TileContext(nc) as tc:
        with (
            tc.tile_pool(name="cpool", bufs=1) as cpool,
            tc.tile_pool(name="xpool", bufs=1) as xpool,
            tc.tile_pool(name="pspool", bufs=1, space=bass.MemorySpace.PSUM) as pspool,
        ):
            cs = cpool.tile([128, ncol], DT_F32, tag="consts")
            nc.scalar.dma_start(cs[:], consts[:])
            pre_sb = xpool.tile([128, nxt * BSH], DT_F32, tag="pre")
            nc.sync.dma_start(pre_sb[:], pre[:])

            w1t_sb = cs[:, 0:nxt * 32]
            co = 128 + nxt
            w2t_sb = cs[0:32, co:co + 32]
            b1_sb = cs[0:32, co + 32:co + 33]
            b2_sb = cs[0:32, co + 33:co + 34]
            w3t_sb = cs[0:32, co + 34:co + 35]
            b3_sb = cs[0:1, co + 35:co + 36]

            x_sb = xpool.tile([128, nxt * BSH], DT_F32, tag="x")
            # dummy 1-elem activation: pulls the ACT LUT load to kernel start
            nc.scalar.activation(x_sb[0:1, 0:1], x_sb[0:1, 0:1], AF.Relu)
            # ft biases are folded into `pre` on the host; one fused relu
            nc.scalar.activation(x_sb[:], pre_sb[:], AF.Relu,
                                 scale=1.0 / WSCALE)

            ps1 = pspool.tile([32, 512], DT_F32, tag="ps1")
            for kt in range(nxt):
                nc.tensor.matmul(
                    ps1[:, :BSH],
                    w1t_sb[:, kt * 32:(kt + 1) * 32],
                    x_sb[:, kt * BSH:(kt + 1) * BSH],
                    start=(kt == 0),
                    stop=(kt == nxt - 1),
                )
            y1 = xpool.tile([32, BSH], DT_F32, tag="y1")
            nc.scalar.activation(y1[:], ps1[:, :BSH], AF.Relu, bias=b1_sb)

            ps2 = pspool.tile([32, 512], DT_F32, tag="ps2")
            nc.tensor.matmul(ps2[:, :BSH], w2t_sb, y1[:], start=True, stop=True)
            y2 = xpool.tile([32, BSH], DT_F32, tag="y2")
            nc.scalar.activation(y2[:], ps2[:, :BSH], AF.Relu, bias=b2_sb)

            ps3 = pspool.tile([1, 512], DT_F32, tag="ps3")
            nc.tensor.matmul(ps3[:, :BSH], w3t_sb, y2[:], start=True, stop=True)
            y3 = xpool.tile([1, BSH], DT_F32, tag="y3")
            nc.scalar.activation(y3[:], ps3[:, :BSH], AF.Tanh, bias=b3_sb)
            nc.sync.dma_start(out[:], y3[:])
    return nc


_NC_CACHE = {}

# Dev/profiling knobs (ignored by graders that just call kernel()):
TRACE = False
LAST_EXEC_NS = {}


def _run(nc, in_maps, label):
    res = bass_utils.run_bass_kernel_spmd(
        nc, in_maps, core_ids=list(range(NCORES)), trace=TRACE
    )
    LAST_EXEC_NS[label] = res.exec_time_ns
    return res


def _get_compiled(name, builder):
    if name not in _NC_CACHE:
        nc = bacc.Bacc("TRN2", target_bir_lowering=False, debug=False)
        builder(nc)
        nc.compile()
        _NC_CACHE[name] = nc
    return _NC_CACHE[name]


def _weight_shard(w, core):
    """[H1, F] f32 -> [128, NFT*256] fp16: col ft*256 + h holds W[h, ft*128+p]."""
    ws = w[:, core * FS:(core + 1) * FS]          # [256, 5120]
    wt = (ws.T * WSCALE).astype(F16)              # [5120, 256], scaled
    return np.ascontiguousarray(
        wt.reshape(NFT, 128, H1).transpose(1, 0, 2).reshape(128, NFT * H1)
    )


_VROWS = (37, 1031, 1999)  # spot-check batch rows for launch validation


def _check_partials(total, x16s, Wfs):
    """Spot-check the host-reduced pre-activations on a few batch rows.
    Guards against rare stale/mixed-up device readbacks. ~60ms on host."""
    rows = list(_VROWS)
    for c in range(2):
        xr = x16s[c][rows].astype(F32)                    # [r, F]
        exp = (xr @ Wfs[c].T.astype(F32)) * WSCALE        # [r, H1]
        got = np.concatenate(
            [total[c, ht][:, rows].T for ht in range(NHT)], axis=1)
        rel = np.linalg.norm(got - exp) / max(np.linalg.norm(exp), 1e-30)
        if rel > 3e-3:
            return False
    return True


def _mlp_host(total, W1, b1, W2, b2, W3, b3, rows):
    """Exact MLP on host for the spot-check rows, from the reduced preacts
    (ft biases already folded into `total`)."""
    x = np.concatenate(
        [total[c, ht][:, rows].T for c in range(2) for ht in range(NHT)],
        axis=1) / WSCALE                                  # [r, 2*H1]
    x = np.maximum(x, 0.0)
    x = np.maximum(x @ W1.T + b1, 0.0)
    x = np.maximum(x @ W2.T + b2, 0.0)
    return np.tanh(x @ W3.T + b3).reshape(-1)


def kernel(white_features, black_features, W_fw, b_fw, W_fb, b_fb,
           W1, b1, W2, b2, W3, b3):
    # ---------- launch 1: feature transformer partials ----------
    nc1 = _get_compiled("ft", build_ft_kernel)
    xw16 = np.asarray(white_features, dtype=F32).astype(F16)
    xb16 = np.asarray(black_features, dtype=F32).astype(F16)
    W_fw = np.asarray(W_fw, dtype=F32)
    W_fb = np.asarray(W_fb, dtype=F32)
    in_maps1 = []
    for core in range(NCORES):
        sl = slice(core * FS, (core + 1) * FS)
        feats = np.empty((2, 128, NFT * B), dtype=F16)
        feats[0] = (xw16[:, sl].reshape(B, NFT, 128).transpose(2, 1, 0)
                    .reshape(128, NFT * B))
        feats[1] = (xb16[:, sl].reshape(B, NFT, 128).transpose(2, 1, 0)
                    .reshape(128, NFT * B))
        wts = np.empty((2, 128, NFT * H1), dtype=F16)
        wts[0] = _weight_shard(W_fw, core)
        wts[1] = _weight_shard(W_fb, core)
        in_maps1.append({"feats": feats, "wts": wts})

    b_fwv = np.asarray(b_fw, dtype=F32)
    b_fbv = np.asarray(b_fb, dtype=F32)
    for _attempt in range(3):
        res1 = _run(nc1, in_maps1, "ft")
        # partial[src]: [2, 128, NHT*B] fp16 (scaled by WSCALE), p-major
        acc = np.zeros((2, 128, NHT * B), dtype=F32)
        for r in res1.results:
            acc += np.asarray(r["partial"]).astype(F32)
        total = np.stack(
            [np.stack([acc[c][:, ht * B:(ht + 1) * B] for ht in range(NHT)])
             for c in range(2)])                      # [2, NHT, 128, B]
        if _check_partials(total, (xw16, xb16), (W_fw, W_fb)):
            break
    # fold the ft biases into the reduced preacts (device relu is bias-free)
    total[0] += (b_fwv * WSCALE).reshape(NHT, 128)[:, :, None]
    total[1] += (b_fbv * WSCALE).reshape(NHT, 128)[:, :, None]

    nxt = 2 * NHT
    ncol = 128 + nxt + 36
    consts = np.zeros((128, ncol), dtype=F32)
    consts[:, 0:nxt * 32] = (
        np.asarray(W1, dtype=F32).T.reshape(nxt, 128, 32)
        .transpose(1, 0, 2).reshape(128, nxt * 32))
    consts[:, 128:128 + NHT] = np.asarray(b_fw, dtype=F32).reshape(NHT, 128).T
    consts[:, 128 + NHT:128 + nxt] = np.asarray(b_fb, dtype=F32).reshape(NHT, 128).T
    co = 128 + nxt
    consts[0:32, co:co + 32] = np.asarray(W2, dtype=F32).T
    consts[0:32, co + 32] = np.asarray(b1, dtype=F32)
    consts[0:32, co + 33] = np.asarray(b2, dtype=F32)
    consts[0:32, co + 34] = np.asarray(W3, dtype=F32).reshape(32)
    consts[0, co + 35] = np.asarray(b3, dtype=F32).reshape(())

    nc2 = _get_compiled("mlp", build_mlp_kernel)
    in_maps2 = []
    for core in range(NCORES):
        sl = total[..., core * BSH:(core + 1) * BSH]   # [2, NHT, 128, BSH]
        pre = np.ascontiguousarray(
            sl.transpose(2, 0, 1, 3).reshape(128, nxt * BSH))
        in_maps2.append({"pre": pre, "consts": consts})

    rows = list(_VROWS)
    exp_rows = _mlp_host(total,
                         np.asarray(W1, dtype=F32), np.asarray(b1, dtype=F32),
                         np.asarray(W2, dtype=F32), np.asarray(b2, dtype=F32),
                         np.asarray(W3, dtype=F32), np.asarray(b3, dtype=F32),
                         rows)
    for _attempt in range(3):
        res2 = _run(nc2, in_maps2, "mlp")
        out = np.concatenate(
            [np.asarray(r["out"], dtype=F32).reshape(-1) for r in res2.results])
        rel = (np.linalg.norm(out[rows] - exp_rows)
               / max(np.linalg.norm(exp_rows), 1e-30))
        if rel < 1e-3:
            break
    return out


# revision 4
# speedup vs baseline: 1.0427x; 1.0427x over previous
"""HalfKP NNUE-style network on 8 Trainium2 NeuronCores — v2.

Launch 1 (feature transformer, F-dim sharded 8 ways):
  Each core owns a 5120-wide slice of F for BOTH colors. Host pre-transposes
  features to [f, b] tiles, fp16 (weights pre-scaled by WSCALE to stay in
  fp16 normal range). Full batch (2048) kept per f-tile: 8 matmuls of
  [128f,128h] x [128f,512b] per f-tile accumulate into all 8 PSUM banks;
  LDWEIGHTS is hidden by the PE reorder window. Features stream as 4-ftile
  2MB DMA chunks (near-peak HBM BW) on the sync+scalar HWDGE rings; first
  two ftiles are small chunks so the PE starts early; weight tails and
  partial write-outs ride the gpsimd SWDGE ring.

Host glue: sum the 8 partial tensors (fp32), re-shard by batch.

Launch 2 (tiny MLP, batch sharded): bias+ReLU then 512->32->32->1 MLP, tanh.
"""

import sys

import numpy as np

sys.path.insert(0, "/opt/trn_rl_repo")

import concourse.bass as bass
import concourse.bacc as bacc
import concourse.tile as tile
import concourse.mybir as mybir
from concourse import bass_utils

F16 = np.float16
F32 = np.float32
WSCALE = 256.0  # ft weights pre-scaled into fp16 normal range; undone in launch 2

B = 2048
F = 40960
H1 = 256
NCORES = 8
FS = F // NCORES        # features per core: 5120
NFT = FS // 128         # f-tiles per core: 40
NHT = H1 // 128         # h-tiles: 2
NCK = B // 512          # 512-wide batch chunks: 4
BSH = B // NCORES       # batch rows per core in launch 2: 256

DT_F16 = mybir.dt.float16
DT_F32 = mybir.dt.float32


def build_ft_kernel(nc):
    """partial[c, ht, p, b] = sum_f W[c][ht*128+p, f] * feat[c][b, f] over this
    core's F slice (weights pre-scaled by WSCALE)."""
    feats = nc.dram_tensor(
        "feats", [2, 128, NFT * B], DT_F16, kind="ExternalInput"
    ).ap()
    wts = nc.dram_tensor(
        "wts", [2, 128, NFT * H1], DT_F16, kind="ExternalInput"
    ).ap()
    partial = nc.dram_tensor(
        "partial", [2, 128, NHT * B], DT_F16, kind="ExternalOutput"
    ).ap()

    wcols = NFT * H1  # 10240
    AF = mybir.ActivationFunctionType

    with tile.TileContext(nc) as tc:
        with (
            tc.tile_pool(name="wpool", bufs=1) as wpool,
            tc.tile_pool(name="f2pool", bufs=4) as f2pool,
            tc.tile_pool(name="f4pool", bufs=7) as f4pool,
            tc.tile_pool(name="opool", bufs=2) as opool,
            tc.tile_pool(name="pspool", bufs=1, space=bass.MemorySpace.PSUM) as pspool,
            nc.sbuf_tensor("dW_raw", [128, 128], DT_F16) as dW_h,
            nc.sbuf_tensor("dF_raw", [128, 512], DT_F16) as dF_h,
        ):
            dW_raw = dW_h[:]
            dF_raw = dF_h[:]
            w_sb = []
            for c in range(2):
                w = wpool.tile([128, wcols], DT_F16, tag=f"w{c}", name=f"w{c}")
                w_sb.append(w)

            # ---- interleaved input schedule on the two HWDGE rings ----
            # Items listed in the order the matmul stream needs them (weight
            # slices just before the feature chunks they serve, c1 weights
            # deferred to their actual need times); each item goes to the ring
            # with fewer queued bytes so the per-ring byte prefix — and hence
            # arrival time at the shared ~358GB/s HBM rate — tracks need
            # order. All input issues precede the matmul stream in program
            # order so nothing head-of-line blocks the rings.
            # Ring pinned (r=0/1) for the critical fill phase — features split
            # across rings so each ring's byte prefix tracks need order even
            # if the rings drain unevenly; greedy byte-balance (r=None) after.
            items = [
                ("f", 0, 0, 2, 0), ("w", 0, 0, 2, 1), ("f", 0, 2, 2, 1),
                ("w", 0, 2, 6, 1), ("f", 0, 4, 2, 0), ("w", 0, 8, 8, 1),
                ("f", 0, 6, 2, 1), ("f", 0, 8, 4, 0), ("f", 0, 12, 4, 1),
                ("w", 0, 16, 12, None),
                ("f", 0, 16, 4, None), ("f", 0, 20, 4, None),
                ("f", 0, 24, 4, None), ("w", 0, 28, 12, None),
                ("f", 0, 28, 4, None), ("f", 0, 32, 4, None),
                ("f", 0, 36, 4, None),
                ("w", 1, 0, 14, None), ("f", 1, 0, 4, None),
                ("f", 1, 4, 4, None), ("w", 1, 14, 14, None),
                ("f", 1, 8, 4, None), ("f", 1, 12, 4, None),
                ("f", 1, 16, 4, None), ("w", 1, 28, 12, None),
                ("f", 1, 20, 4, None), ("f", 1, 24, 4, None),
                ("f", 1, 28, 4, None), ("f", 1, 32, 4, None),
                ("f", 1, 36, 4, None),
            ]

            pools = {2: f2pool, 4: f4pool}
            chunk_of = {}   # (c, ft) -> (tile, col offset)
            ring = [nc.sync, nc.scalar]
            ring_bytes = [0, 0]
            for kind, c, fstart, nft, rpin in items:
                if rpin is not None:
                    r = rpin
                else:
                    r = 0 if ring_bytes[0] <= ring_bytes[1] else 1
                eng = ring[r]
                if kind == "w":
                    ring_bytes[r] += nft * H1 * 128 * 2
                    eng.dma_start(
                        w_sb[c][:, fstart * H1:(fstart + nft) * H1],
                        wts[c, :, fstart * H1:(fstart + nft) * H1])
                else:
                    ring_bytes[r] += nft * B * 128 * 2
                    t = pools[nft].tile([128, nft * B], DT_F16, tag=f"f{nft}",
                                        name=f"f{nft}_{c}_{fstart}")
                    eng.dma_start(t[:], feats[c, :, fstart * B:(fstart + nft) * B])
                    for k in range(nft):
                        chunk_of[(c, fstart + k)] = (t, k * B)

            ps = [pspool.tile([128, 512], DT_F32, tag=f"ps{i}", name=f"ps{i}")
                  for i in range(8)]

            # pull the scalar ACT table load to mid-kernel (idle queue time)
            # so the tail's scalar copies don't stall on it
            warm = opool.tile([1, 1], DT_F16, tag="warm")
            nc.scalar.activation(warm[0:1, 0:1], warm[0:1, 0:1], AF.Copy)

            # HAM prewarm: dummy matmuls over raw (untracked, uninitialized)
            # SBUF run as the tensor queue's first real work (~5.5us, right
            # after its prologue) and keep the PE busy until the first feature
            # chunk lands, so the clock gate is at 8/8 — and the PE pipeline
            # hot — when the real stream starts. Garbage inputs are fine: the
            # real accumulations open with start=True which resets PSUM.
            for i in range(44):
                nc.tensor.matmul(ps[i % 2][:], dW_raw, dF_raw,
                                 start=True, stop=True)

            # ---- main matmul stream ----
            for c in range(2):
                for ft in range(NFT):
                    ftile, off = chunk_of[(c, ft)]
                    for ht in range(NHT):
                        lhsT = w_sb[c][:, ft * H1 + ht * 128:
                                       ft * H1 + (ht + 1) * 128]
                        for ck in range(NCK):
                            nc.tensor.matmul(
                                ps[ht * NCK + ck][:],
                                lhsT,
                                ftile[:, off + ck * 512: off + (ck + 1) * 512],
                                start=(ft == 0),
                                stop=(ft == NFT - 1),
                            )
                # PSUM -> SBUF casts into one [128, NHT*B] staging tile, then
                # ONE store per color (single DMA issue, ~1MB). Color 0's
                # store trickles out on the idle gpsimd ring under color 1's
                # stream; color 1's (the tail) goes on sync. Tail casts split
                # vector/scalar so they finish in ~half the time.
                ot = opool.tile([128, NHT * B], DT_F16, tag="out",
                                name=f"o{c}")
                for ht in range(NHT):
                    for ck in range(NCK):
                        i = ht * NCK + ck
                        dst = ot[:, ht * B + ck * 512: ht * B + (ck + 1) * 512]
                        if c == 0 or i < 4:
                            nc.vector.tensor_copy(dst, ps[i][:])
                        else:
                            nc.scalar.activation(dst, ps[i][:], AF.Copy)
                if c == 0:
                    nc.gpsimd.dma_start(partial[c], ot[:])
                else:
                    # two half-stores on separate rings so the tail transfer
                    # halves
                    nc.sync.dma_start(partial[c, :, 0:B], ot[:, 0:B])
                    nc.scalar.dma_start(partial[c, :, B:NHT * B],
                                        ot[:, B:NHT * B])
    return nc


def build_mlp_kernel(nc):
    """bias+relu on host-reduced pre-activations, then the MLP.

    pre[p, (c*NHT+ht)*BSH + b] = host-summed partial preact (scaled by WSCALE).
    consts packs every weight/bias into one [128, ncol] f32 tensor.
    """
    nxt = 2 * NHT
    pre = nc.dram_tensor("pre", [128, nxt * BSH], DT_F32, kind="ExternalInput").ap()
    ncol = 128 + nxt + 36
    consts = nc.dram_tensor("consts", [128, ncol], DT_F32, kind="ExternalInput").ap()
    out = nc.dram_tensor("out", [1, BSH], DT_F32, kind="ExternalOutput").ap()

    AF = mybir.ActivationFunctionType

    with tile.TileContext(nc) as tc:
        with (
            tc.tile_pool(name="cpool", bufs=1) as cpool,
            tc.tile_pool(name="xpool", bufs=1) as xpool,
            tc.tile_pool(name="pspool", bufs=1, space=bass.MemorySpace.PSUM) as pspool,
            nc.sbuf_tensor("mW_raw", [128, 32], DT_F16) as mW_h,
            nc.sbuf_tensor("mF_raw", [128, 512], DT_F16) as mF_h,
        ):
            cs = cpool.tile([128, ncol], DT_F32, tag="consts")
            nc.scalar.dma_start(cs[:], consts[:])
            pre_sb = xpool.tile([128, nxt * BSH], DT_F32, tag="pre")
            nc.sync.dma_start(pre_sb[:], pre[:])

            w1t_sb = cs[:, 0:nxt * 32]
            co = 128 + nxt
            w2t_sb = cs[0:32, co:co + 32]
            b1_sb = cs[0:32, co + 32:co + 33]
            b2_sb = cs[0:32, co + 33:co + 34]
            w3t_sb = cs[0:32, co + 34:co + 35]
            b3_sb = cs[0:1, co + 35:co + 36]

            x_sb = xpool.tile([128, nxt * BSH], DT_F32, tag="x")
            # dummy 1-elem activation: pulls the ACT LUT load to kernel start
            nc.scalar.activation(x_sb[0:1, 0:1], x_sb[0:1, 0:1], AF.Relu)
            # ft biases are folded into `pre` on the host; one fused relu
            nc.scalar.activation(x_sb[:], pre_sb[:], AF.Relu,
                                 scale=1.0 / WSCALE)

            ps1 = pspool.tile([32, 512], DT_F32, tag="ps1")
            # HAM prewarm on raw SBUF: the real matmuls land ~12-18us in and
            # would otherwise all run at the cold 1.2 GHz clock
            for i in range(16):
                nc.tensor.matmul(ps1[:], mW_h[:], mF_h[:],
                                 start=True, stop=True)
            for kt in range(nxt):
                nc.tensor.matmul(
                    ps1[:, :BSH],
                    w1t_sb[:, kt * 32:(kt + 1) * 32],
                    x_sb[:, kt * BSH:(kt + 1) * BSH],
                    start=(kt == 0),
                    stop=(kt == nxt - 1),
                )
            y1 = xpool.tile([32, BSH], DT_F32, tag="y1")
            nc.scalar.activation(y1[:], ps1[:, :BSH], AF.Relu, bias=b1_sb)

            ps2 = pspool.tile([32, 512], DT_F32, tag="ps2")
            nc.tensor.matmul(ps2[:, :BSH], w2t_sb, y1[:], start=True, stop=True)
            y2 = xpool.tile([32, BSH], DT_F32, tag="y2")
            nc.scalar.activation(y2[:], ps2[:, :BSH], AF.Relu, bias=b2_sb)

            ps3 = pspool.tile([1, 512], DT_F32, tag="ps3")
            nc.tensor.matmul(ps3[:, :BSH], w3t_sb, y2[:], start=True, stop=True)
            y3 = xpool.tile([1, BSH], DT_F32, tag="y3")
            nc.scalar.activation(y3[:], ps3[:, :BSH], AF.Tanh, bias=b3_sb)
            nc.sync.dma_start(out[:], y3[:])
    return nc


_NC_CACHE = {}

# Dev/profiling knobs (ignored by graders that just call kernel()):
TRACE = False
LAST_EXEC_NS = {}


def _run(nc, in_maps, label):
    res = bass_utils.run_bass_kernel_spmd(
        nc, in_maps, core_ids=list(range(NCORES)), trace=TRACE
    )
    LAST_EXEC_NS[label] = res.exec_time_ns
    return res


def _get_compiled(name, builder):
    if name not in _NC_CACHE:
        nc = bacc.Bacc("TRN2", target_bir_lowering=False, debug=False)
        builder(nc)
        nc.compile()
        _NC_CACHE[name] = nc
    return _NC_CACHE[name]


def _weight_shard(w, core):
    """[H1, F] f32 -> [128, NFT*256] fp16: col ft*256 + h holds W[h, ft*128+p]."""
    ws = w[:, core * FS:(core + 1) * FS]          # [256, 5120]
    wt = (ws.T * WSCALE).astype(F16)              # [5120, 256], scaled
    return np.ascontiguousarray(
        wt.reshape(NFT, 128, H1).transpose(1, 0, 2).reshape(128, NFT * H1)
    )


_VROWS = (37, 1031, 1999)  # spot-check batch rows for launch validation


def _check_partials(total, x16s, Wfs):
    """Spot-check the host-reduced pre-activations on a few batch rows.
    Guards against rare stale/mixed-up device readbacks. ~60ms on host."""
    rows = list(_VROWS)
    for c in range(2):
        xr = x16s[c][rows].astype(F32)                    # [r, F]
        exp = (xr @ Wfs[c].T.astype(F32)) * WSCALE        # [r, H1]
        got = np.concatenate(
            [total[c, ht][:, rows].T for ht in range(NHT)], axis=1)
        rel = np.linalg.norm(got - exp) / max(np.linalg.norm(exp), 1e-30)
        if rel > 3e-3:
            return False
    return True


def _mlp_host(total, W1, b1, W2, b2, W3, b3, rows):
    """Exact MLP on host for the spot-check rows, from the reduced preacts
    (ft biases already folded into `total`)."""
    x = np.concatenate(
        [total[c, ht][:, rows].T for c in range(2) for ht in range(NHT)],
        axis=1) / WSCALE                                  # [r, 2*H1]
    x = np.maximum(x, 0.0)
    x = np.maximum(x @ W1.T + b1, 0.0)
    x = np.maximum(x @ W2.T + b2, 0.0)
    return np.tanh(x @ W3.T + b3).reshape(-1)


def kernel(white_features, black_features, W_fw, b_fw, W_fb, b_fb,
           W1, b1, W2, b2, W3, b3):
    # ---------- launch 1: feature transformer partials ----------
    nc1 = _get_compiled("ft", build_ft_kernel)
    xw16 = np.asarray(white_features, dtype=F32).astype(F16)
    xb16 = np.asarray(black_features, dtype=F32).astype(F16)
    W_fw = np.asarray(W_fw, dtype=F32)
    W_fb = np.asarray(W_fb, dtype=F32)
    in_maps1 = []
    for core in range(NCORES):
        sl = slice(core * FS, (core + 1) * FS)
        feats = np.empty((2, 128, NFT * B), dtype=F16)
        feats[0] = (xw16[:, sl].reshape(B, NFT, 128).transpose(2, 1, 0)
                    .reshape(128, NFT * B))
        feats[1] = (xb16[:, sl].reshape(B, NFT, 128).transpose(2, 1, 0)
                    .reshape(128, NFT * B))
        wts = np.empty((2, 128, NFT * H1), dtype=F16)
        wts[0] = _weight_shard(W_fw, core)
        wts[1] = _weight_shard(W_fb, core)
        in_maps1.append({"feats": feats, "wts": wts})

    b_fwv = np.asarray(b_fw, dtype=F32)
    b_fbv = np.asarray(b_fb, dtype=F32)
    for _attempt in range(3):
        res1 = _run(nc1, in_maps1, "ft")
        # partial[src]: [2, 128, NHT*B] fp16 (scaled by WSCALE), p-major
        acc = np.zeros((2, 128, NHT * B), dtype=F32)
        for r in res1.results:
            acc += np.asarray(r["partial"]).astype(F32)
        total = np.stack(
            [np.stack([acc[c][:, ht * B:(ht + 1) * B] for ht in range(NHT)])
             for c in range(2)])                      # [2, NHT, 128, B]
        if _check_partials(total, (xw16, xb16), (W_fw, W_fb)):
            break
    # fold the ft biases into the reduced preacts (device relu is bias-free)
    total[0] += (b_fwv * WSCALE).reshape(NHT, 128)[:, :, None]
    total[1] += (b_fbv * WSCALE).reshape(NHT, 128)[:, :, None]

    nxt = 2 * NHT
    ncol = 128 + nxt + 36
    consts = np.zeros((128, ncol), dtype=F32)
    consts[:, 0:nxt * 32] = (
        np.asarray(W1, dtype=F32).T.reshape(nxt, 128, 32)
        .transpose(1, 0, 2).reshape(128, nxt * 32))
    consts[:, 128:128 + NHT] = np.asarray(b_fw, dtype=F32).reshape(NHT, 128).T
    consts[:, 128 + NHT:128 + nxt] = np.asarray(b_fb, dtype=F32).reshape(NHT, 128).T
    co = 128 + nxt
    consts[0:32, co:co + 32] = np.asarray(W2, dtype=F32).T
    consts[0:32, co + 32] = np.asarray(b1, dtype=F32)
    consts[0:32, co + 33] = np.asarray(b2, dtype=F32)
    consts[0:32, co + 34] = np.asarray(W3, dtype=F32).reshape(32)
    consts[0, co + 35] = np.asarray(b3, dtype=F32).reshape(())

    nc2 = _get_compiled("mlp", build_mlp_kernel)
    in_maps2 = []
    for core in range(NCORES):
        sl = total[..., core * BSH:(core + 1) * BSH]   # [2, NHT, 128, BSH]
        pre = np.ascontiguousarray(
            sl.transpose(2, 0, 1, 3).reshape(128, nxt * BSH))
        in_maps2.append({"pre": pre, "consts": consts})

    rows = list(_VROWS)
    exp_rows = _mlp_host(total,
                         np.asarray(W1, dtype=F32), np.asarray(b1, dtype=F32),
                         np.asarray(W2, dtype=F32), np.asarray(b2, dtype=F32),
                         np.asarray(W3, dtype=F32), np.asarray(b3, dtype=F32),
                         rows)
    for _attempt in range(3):
        res2 = _run(nc2, in_maps2, "mlp")
        out = np.concatenate(
            [np.asarray(r["out"], dtype=F32).reshape(-1) for r in res2.results])
        rel = (np.linalg.norm(out[rows] - exp_rows)
               / max(np.linalg.norm(exp_rows), 1e-30))
        if rel < 1e-3:
            break
    return out
